# revision 1
# baseline (speedup 1.0000x reference)
"""MoE dense all-experts (GPT-OSS Experts forward) on 8 Trainium2 NeuronCores.

Expert-parallel sharding: core e holds expert e's weights and computes its
weighted contribution

    partial_e[t, h] = w[t, e] * ((up + 1) * silu(1.702 * gate) @ down_e.T + db_e)

with [gate | up] = hs @ gup_e + bias (the host de-interleaves gup's even/odd
columns so gate/up become contiguous halves). The token dimension is processed
in chunks; each chunk's partials are summed across the 8 cores with a
ReduceScatter that overlaps the next chunk's compute, and the host reassembles
the T-sharded RS outputs into the full [T, H] result.

Matmuls run in float32r (fp32 data on the PE at ~1 cycle/row, TF32-like
precision; end-to-end relative error ~2e-4). Stage 1 computes [f, t] tiles
(gate pass feeding the ScalarE Silu LUT, then up pass fused with the silu
output via scalar_tensor_tensor into act[i, t]); stage 2 computes out[t, h]
with act as the stationary operand. The down-bias + routing-weight epilogue
runs on the VectorE: out = (psum * w[t]) + w[t]*db[h], with the rank-1 w*db
tile built from a partition-broadcast copy of db.
"""
import sys
if '/opt/trn_rl_repo' not in sys.path:
    sys.path.insert(0, '/opt/trn_rl_repo')
import numpy as np

E, H, I, T = 8, 1024, 1024, 4096
N_CORES = 8
CHUNKS = [512] * 6 + [384, 384, 256]
KC = H // 128          # contraction chunks (H == I == 1024)
NJ = I // 128          # gate/up row tiles
TCMAX = max(CHUNKS)

_CACHE = {}


def _build():
    import concourse.bacc as bacc
    import concourse.tile as tile
    import concourse.mybir as mybir
    f32 = mybir.dt.float32
    f32r = mybir.dt.float32r
    AF = mybir.ActivationFunctionType
    ALU = mybir.AluOpType

    nc = bacc.Bacc("TRN2", target_bir_lowering=False, debug=False,
                   enable_asserts=False, num_devices=N_CORES)
    hsT = nc.dram_tensor("hsT", [H, T], f32r, kind="ExternalInput").ap()
    gup = nc.dram_tensor("gup", [H, 2 * I], f32r, kind="ExternalInput").ap()
    gb = nc.dram_tensor("gb", [128, NJ], f32, kind="ExternalInput").ap()
    ub = nc.dram_tensor("ub", [128, NJ], f32, kind="ExternalInput").ap()
    dwT = nc.dram_tensor("dwT", [I, H], f32r, kind="ExternalInput").ap()
    db = nc.dram_tensor("db", [1, H], f32, kind="ExternalInput").ap()
    wt = nc.dram_tensor("wt", [128, T // 128], f32, kind="ExternalInput").ap()
    osh = nc.dram_tensor("osh", [T // N_CORES, H], f32, kind="ExternalOutput").ap()
    otail = nc.dram_tensor("otail", [CHUNKS[-1], H], f32, kind="ExternalOutput").ap()

    with tile.TileContext(nc) as tc_:
        with tc_.tile_pool(name="wpool", bufs=1) as wpool, \
             tc_.tile_pool(name="hpool", bufs=2) as hpool, \
             tc_.tile_pool(name="apool", bufs=2) as apool, \
             tc_.tile_pool(name="spool", bufs=8) as spool, \
             tc_.tile_pool(name="opool", bufs=3) as opool, \
             tc_.tile_pool(name="bpool", bufs=4) as bpool, \
             tc_.tile_pool(name="dpool", bufs=2, space="DRAM") as dpool, \
             tc_.tile_pool(name="ps1", bufs=2, space="PSUM") as ps1, \
             tc_.tile_pool(name="ps2", bufs=3, space="PSUM") as ps2:

            gup_r = wpool.tile([128, KC * 2 * I], f32r)
            dwT_r = wpool.tile([128, KC * H], f32r)
            gb_r = wpool.tile([128, NJ], f32)
            ub_r = wpool.tile([128, NJ], f32)
            db_f = wpool.tile([1, H], f32)
            db_bc = wpool.tile([128, H], f32)
            w_r = wpool.tile([128, T // 128], f32)

            # DMA order matches consumption order: tiny bias/route tensors,
            # then per-kc (hs0, gate) pairs so the k-accumulation is DMA-paced,
            # then the up half, the chunk-1 token prefetch, and the down weights.
            nc.sync.dma_start(gb_r[:], gb[:])
            nc.sync.dma_start(ub_r[:], ub[:])
            nc.sync.dma_start(db_f[:], db[:])
            nc.sync.dma_start(w_r[:], wt[:])
            hs0 = hpool.tile([128, KC * TCMAX], f32r, tag="hs")
            for kc in range(KC):
                nc.sync.dma_start(hs0[:, kc*TCMAX:kc*TCMAX + CHUNKS[0]],
                                  hsT[kc*128:(kc+1)*128, 0:CHUNKS[0]])
                nc.sync.dma_start(gup_r[:, kc*2*I : kc*2*I + I],
                                  gup[kc*128:(kc+1)*128, 0:I])
            for kc in range(KC):
                nc.sync.dma_start(gup_r[:, kc*2*I + I : (kc+1)*2*I],
                                  gup[kc*128:(kc+1)*128, I:2*I])
            hs1 = hpool.tile([128, KC * TCMAX], f32r, tag="hs")
            nc.sync.dma_start(
                hs1[:].rearrange("p (kc t) -> p kc t", t=TCMAX)[:, :, 0:CHUNKS[1]],
                hsT[:, CHUNKS[0]:CHUNKS[0] + CHUNKS[1]].rearrange("(kc p) t -> p kc t", p=128))
            for kc in range(KC):
                nc.sync.dma_start(dwT_r[:, kc*H:(kc+1)*H], dwT[kc*128:(kc+1)*128, :])
            nc.gpsimd.partition_broadcast(db_bc[:], db_f[:])

            t_off = 0
            o_off = 0
            for c, TC in enumerate(CHUNKS):
                NTT = TC // 128
                OC = TC // N_CORES
                if c == 0:
                    hs_r = hs0
                elif c == 1:
                    hs_r = hs1
                else:
                    hs_r = hpool.tile([128, KC * TCMAX], f32r, tag="hs")
                    nc.sync.dma_start(
                        hs_r[:].rearrange("p (kc t) -> p kc t", t=TCMAX)[:, :, 0:TC],
                        hsT[:, t_off:t_off + TC].rearrange("(kc p) t -> p kc t", p=128))

                act_r = apool.tile([128, NJ * TCMAX], f32r, tag="act")
                s2s = []
                for j in range(NJ):     # gate pass
                    pg = ps1.tile([128, TC], f32, tag="pg")
                    for kc in range(KC):
                        nc.tensor.matmul(pg[:], gup_r[:, kc*2*I + j*128 : kc*2*I + (j+1)*128],
                                         hs_r[:, kc*TCMAX:kc*TCMAX + TC],
                                         start=(kc == 0), stop=(kc == KC - 1))
                    s2 = spool.tile([128, TCMAX], f32, tag="s2")
                    nc.scalar.activation(s2[:, :TC], pg[:], AF.Silu,
                                         bias=gb_r[:, j:j+1], scale=1.702)
                    s2s.append(s2)
                for j in range(NJ):     # up pass: act = (up + ub + 1) * silu_out
                    pu = ps1.tile([128, TC], f32, tag="pu")
                    for kc in range(KC):
                        nc.tensor.matmul(pu[:], gup_r[:, kc*2*I + I + j*128 : kc*2*I + I + (j+1)*128],
                                         hs_r[:, kc*TCMAX:kc*TCMAX + TC],
                                         start=(kc == 0), stop=(kc == KC - 1))
                    nc.vector.scalar_tensor_tensor(act_r[:, j*TCMAX:j*TCMAX + TC], pu[:],
                                                   ub_r[:, j:j+1], s2s[j][:, :TC],
                                                   op0=ALU.add, op1=ALU.mult)

                last = (c == len(CHUNKS) - 1)
                if not last:
                    bin_ = dpool.tile([TCMAX, H], f32, tag="bi")
                    bout = dpool.tile([TCMAX // N_CORES, H], f32, tag="bo")
                for tt in range(NTT):
                    gt = (t_off // 128) + tt
                    wcol = w_r[:, gt:gt+1]
                    ot = opool.tile([128, H], f32, tag="ot")
                    for hh in range(H // 512):
                        dbw = bpool.tile([128, 512], f32, tag="dbw")
                        nc.vector.tensor_scalar_mul(dbw[:], db_bc[:, hh*512:(hh+1)*512], wcol)
                        p2 = ps2.tile([128, 512], f32, tag="p2")
                        for ic in range(KC):
                            nc.tensor.matmul(p2[:], act_r[:, ic*TCMAX + tt*128 : ic*TCMAX + (tt+1)*128],
                                             dwT_r[:, ic*H + hh*512 : ic*H + (hh+1)*512],
                                             start=(ic == 0), stop=(ic == KC - 1))
                        nc.vector.scalar_tensor_tensor(ot[:, hh*512:(hh+1)*512], p2[:], wcol,
                                                       dbw[:], op0=ALU.mult, op1=ALU.add)
                    if last:
                        # final chunk: ship per-core partials; the host sums
                        # them in fp32 so the device tail ends at the prior RS
                        nc.sync.dma_start(otail[tt*128:(tt+1)*128, :], ot[:])
                    else:
                        nc.sync.dma_start(bin_[tt*128:(tt+1)*128, :], ot[:])
                if not last:
                    nc.gpsimd.collective_compute(
                        "ReduceScatter", ALU.add,
                        replica_groups=[list(range(N_CORES))],
                        ins=[bin_[:TC, :].opt()], outs=[bout[:OC, :].opt()])
                    nc.sync.dma_start(osh[o_off:o_off + OC, :], bout[:OC, :])
                t_off += TC
                o_off += OC
    nc.compile()
    return nc


def _get_nc():
    if 'nc' not in _CACHE:
        _CACHE['nc'] = _build()
    return _CACHE['nc']


def _make_in_maps(hidden_states, routing_weights, gate_up_proj, gate_up_proj_bias,
                  down_proj, down_proj_bias):
    hs = np.ascontiguousarray(np.asarray(hidden_states, dtype=np.float32))
    rw = np.asarray(routing_weights, dtype=np.float32)
    gupw = np.asarray(gate_up_proj, dtype=np.float32)
    gupb = np.asarray(gate_up_proj_bias, dtype=np.float32)
    dw = np.asarray(down_proj, dtype=np.float32)
    dbias = np.asarray(down_proj_bias, dtype=np.float32)
    hsT = np.ascontiguousarray(hs.T)
    in_maps = []
    for e in range(N_CORES):
        g = gupw[e]
        gup_de = np.concatenate([g[:, 0::2], g[:, 1::2]], axis=1)
        in_maps.append({
            "hsT": hsT,
            "gup": np.ascontiguousarray(gup_de),
            # silu(1.702*(x + b)) = silu(1.702*x + 1.702*b); the 1/1.702 glu
            # scale is folded into dwT below.
            "gb": np.ascontiguousarray((1.702 * gupb[e, 0::2]).reshape(NJ, 128).T),
            "ub": np.ascontiguousarray((gupb[e, 1::2] + 1.0).reshape(NJ, 128).T),
            "dwT": np.ascontiguousarray(dw[e].T / np.float32(1.702)),
            "db": np.ascontiguousarray(dbias[e][None, :]),
            "wt": np.ascontiguousarray(rw[:, e].reshape(T // 128, 128).T),
        })
    return in_maps


def _assemble(results):
    out = np.empty((T, H), dtype=np.float32)
    t_off = 0
    o_off = 0
    for TC in CHUNKS[:-1]:
        OC = TC // N_CORES
        for r in range(N_CORES):
            out[t_off + r*OC : t_off + (r+1)*OC, :] = results[r]["osh"][o_off:o_off + OC, :]
        t_off += TC
        o_off += OC
    out[t_off:, :] = np.sum([results[r]["otail"] for r in range(N_CORES)], axis=0)
    return out


def kernel(hidden_states, routing_weights, gate_up_proj, gate_up_proj_bias,
           down_proj, down_proj_bias):
    from concourse import bass_utils
    in_maps = _make_in_maps(hidden_states, routing_weights, gate_up_proj,
                            gate_up_proj_bias, down_proj, down_proj_bias)
    nc = _get_nc()
    try:
        res = bass_utils.run_bass_kernel_spmd(nc, in_maps, core_ids=list(range(N_CORES)))
    except Exception:
        # One retry in case a previous process left a core wedged.
        res = bass_utils.run_bass_kernel_spmd(nc, in_maps, core_ids=list(range(N_CORES)))
    return _assemble(res.results)



# revision 2
# speedup vs baseline: 1.2733x; 1.2733x over previous
"""MoE dense all-experts (GPT-OSS Experts forward) on 8 Trainium2 NeuronCores.

Expert-parallel sharding: core e holds expert e's weights and computes its
weighted contribution

    partial_e[t, h] = w[t, e] * ((up + 1) * silu(1.702 * gate) @ down_e.T)

with [gate | up] = hs @ gup_e + bias (the host de-interleaves gup's even/odd
columns so gate/up become contiguous halves). Each core writes its full [T, H]
partial to DRAM; the host sums the 8 partials and adds the routing-weighted
down-bias term (rw @ db) itself, so the device runs no collective at all.

All matmul operands are bf16 (PSUM accumulation stays fp32): bf16 stationary
tiles get the compiler's automatic Fast Weight Load (4-byte weight reads via 4
XBUSes), so the per-matmul LDWEIGHTS cost (~208 ns at fp32r) drops to ~53 ns
and hides under the 512-row moving streams. End-to-end relative error vs the
fp32 reference is ~3.6e-3 (numpy-simulated and HW-verified).

Stage 1 computes [f, t] tiles (gate pass feeding the ScalarE Silu LUT, then up
pass fused with the silu output via scalar_tensor_tensor into bf16 act[i, t]);
stage 2 computes out[t, h] with act as the stationary operand and a VectorE
epilogue out = psum * w[t]. Token chunks are sized [256, 512*7, 256]: the small
first chunk plus per-(kc, 512-col) weight DMAs shrink the head stall before the
first gate chain, and the small last chunk shrinks the output-DMA tail.
"""
import sys
if '/opt/trn_rl_repo' not in sys.path:
    sys.path.insert(0, '/opt/trn_rl_repo')
import numpy as np
import ml_dtypes

E, H, I, T = 8, 1024, 1024, 4096
N_CORES = 8
CHUNKS = [256] + [512] * 7 + [256]
KC = H // 128          # contraction chunks (H == I == 1024)
NJ = I // 128          # gate/up row tiles
TCMAX = max(CHUNKS)

_CACHE = {}


def _build():
    import concourse.bacc as bacc
    import concourse.tile as tile
    import concourse.mybir as mybir
    f32 = mybir.dt.float32
    bf16 = mybir.dt.bfloat16
    AF = mybir.ActivationFunctionType
    ALU = mybir.AluOpType

    nc = bacc.Bacc("TRN2", target_bir_lowering=False, debug=False,
                   enable_asserts=False, num_devices=N_CORES)
    hsT = nc.dram_tensor("hsT", [H, T], bf16, kind="ExternalInput").ap()
    gup = nc.dram_tensor("gup", [H, 2 * I], bf16, kind="ExternalInput").ap()
    gb = nc.dram_tensor("gb", [128, NJ], f32, kind="ExternalInput").ap()
    ub = nc.dram_tensor("ub", [128, NJ], f32, kind="ExternalInput").ap()
    dwT = nc.dram_tensor("dwT", [I, H], bf16, kind="ExternalInput").ap()
    wt = nc.dram_tensor("wt", [128, T // 128], f32, kind="ExternalInput").ap()
    outp = nc.dram_tensor("outp", [T, H], f32, kind="ExternalOutput").ap()

    with tile.TileContext(nc) as tc_:
        with tc_.tile_pool(name="wpool", bufs=1) as wpool, \
             tc_.tile_pool(name="hpool", bufs=2) as hpool, \
             tc_.tile_pool(name="apool", bufs=2) as apool, \
             tc_.tile_pool(name="spool", bufs=8) as spool, \
             tc_.tile_pool(name="opool", bufs=3) as opool, \
             tc_.tile_pool(name="ps1", bufs=2, space="PSUM") as ps1, \
             tc_.tile_pool(name="ps2", bufs=3, space="PSUM") as ps2:

            gup_r = wpool.tile([128, KC * 2 * I], bf16)
            dwT_r = wpool.tile([128, KC * H], bf16)
            gb_r = wpool.tile([128, NJ], f32)
            ub_r = wpool.tile([128, NJ], f32)
            w_r = wpool.tile([128, T // 128], f32)

            # DMA order matches consumption order: tiny bias/route tensors,
            # then per-kc (hs0, gate-block) pairs so the first gate chains are
            # DMA-paced, then the remaining gate/up blocks, the chunk-1 token
            # prefetch, and the down weights.
            nc.sync.dma_start(gb_r[:], gb[:])
            nc.sync.dma_start(ub_r[:], ub[:])
            nc.sync.dma_start(w_r[:], wt[:])
            hs0 = hpool.tile([128, KC * TCMAX], bf16, tag="hs")
            for kc in range(KC):
                nc.sync.dma_start(hs0[:, kc*TCMAX:kc*TCMAX + CHUNKS[0]],
                                  hsT[kc*128:(kc+1)*128, 0:CHUNKS[0]])
                nc.sync.dma_start(gup_r[:, kc*2*I : kc*2*I + 512],
                                  gup[kc*128:(kc+1)*128, 0:512])
            for kc in range(KC):
                nc.sync.dma_start(gup_r[:, kc*2*I + 512 : kc*2*I + I],
                                  gup[kc*128:(kc+1)*128, 512:I])
            for kc in range(KC):
                nc.sync.dma_start(gup_r[:, kc*2*I + I : kc*2*I + I + 512],
                                  gup[kc*128:(kc+1)*128, I:I + 512])
            for kc in range(KC):
                nc.sync.dma_start(gup_r[:, kc*2*I + I + 512 : (kc+1)*2*I],
                                  gup[kc*128:(kc+1)*128, I + 512:2*I])
            hs1 = hpool.tile([128, KC * TCMAX], bf16, tag="hs")
            nc.sync.dma_start(
                hs1[:].rearrange("p (kc t) -> p kc t", t=TCMAX)[:, :, 0:CHUNKS[1]],
                hsT[:, CHUNKS[0]:CHUNKS[0] + CHUNKS[1]].rearrange("(kc p) t -> p kc t", p=128))
            for kc in range(KC):
                nc.sync.dma_start(dwT_r[:, kc*H:(kc+1)*H], dwT[kc*128:(kc+1)*128, :])

            t_off = 0
            for c, TC in enumerate(CHUNKS):
                NTT = TC // 128
                if c == 0:
                    hs_r = hs0
                elif c == 1:
                    hs_r = hs1
                else:
                    hs_r = hpool.tile([128, KC * TCMAX], bf16, tag="hs")
                    nc.sync.dma_start(
                        hs_r[:].rearrange("p (kc t) -> p kc t", t=TCMAX)[:, :, 0:TC],
                        hsT[:, t_off:t_off + TC].rearrange("(kc p) t -> p kc t", p=128))

                act_r = apool.tile([128, NJ * TCMAX], bf16, tag="act")
                s2s = []
                for j in range(NJ):     # gate pass
                    pg = ps1.tile([128, TC], f32, tag="pg")
                    for kc in range(KC):
                        nc.tensor.matmul(pg[:], gup_r[:, kc*2*I + j*128 : kc*2*I + (j+1)*128],
                                         hs_r[:, kc*TCMAX:kc*TCMAX + TC],
                                         start=(kc == 0), stop=(kc == KC - 1))
                    s2 = spool.tile([128, TCMAX], f32, tag="s2")
                    nc.scalar.activation(s2[:, :TC], pg[:], AF.Silu,
                                         bias=gb_r[:, j:j+1], scale=1.702)
                    s2s.append(s2)
                for j in range(NJ):     # up pass: act = (up + ub + 1) * silu_out
                    pu = ps1.tile([128, TC], f32, tag="pu")
                    for kc in range(KC):
                        nc.tensor.matmul(pu[:], gup_r[:, kc*2*I + I + j*128 : kc*2*I + I + (j+1)*128],
                                         hs_r[:, kc*TCMAX:kc*TCMAX + TC],
                                         start=(kc == 0), stop=(kc == KC - 1))
                    nc.vector.scalar_tensor_tensor(act_r[:, j*TCMAX:j*TCMAX + TC], pu[:],
                                                   ub_r[:, j:j+1], s2s[j][:, :TC],
                                                   op0=ALU.add, op1=ALU.mult)

                for tt in range(NTT):
                    gt = (t_off // 128) + tt
                    wcol = w_r[:, gt:gt+1]
                    ot = opool.tile([128, H], f32, tag="ot")
                    for hh in range(H // 512):
                        p2 = ps2.tile([128, 512], f32, tag="p2")
                        for ic in range(KC):
                            nc.tensor.matmul(p2[:], act_r[:, ic*TCMAX + tt*128 : ic*TCMAX + (tt+1)*128],
                                             dwT_r[:, ic*H + hh*512 : ic*H + (hh+1)*512],
                                             start=(ic == 0), stop=(ic == KC - 1))
                        nc.vector.tensor_scalar_mul(ot[:, hh*512:(hh+1)*512], p2[:], wcol)
                    nc.sync.dma_start(outp[t_off + tt*128 : t_off + (tt+1)*128, :], ot[:])
                t_off += TC
    nc.compile()
    return nc


def _get_nc():
    if 'nc' not in _CACHE:
        _CACHE['nc'] = _build()
    return _CACHE['nc']


def _make_in_maps(hidden_states, routing_weights, gate_up_proj, gate_up_proj_bias,
                  down_proj, down_proj_bias):
    bf = ml_dtypes.bfloat16
    hs = np.asarray(hidden_states, dtype=np.float32)
    rw = np.asarray(routing_weights, dtype=np.float32)
    gupw = np.asarray(gate_up_proj, dtype=np.float32)
    gupb = np.asarray(gate_up_proj_bias, dtype=np.float32)
    dw = np.asarray(down_proj, dtype=np.float32)
    hsT = np.ascontiguousarray(hs.T).astype(bf)
    in_maps = []
    for e in range(N_CORES):
        g = gupw[e]
        gup_de = np.concatenate([g[:, 0::2], g[:, 1::2]], axis=1)
        in_maps.append({
            "hsT": hsT,
            "gup": np.ascontiguousarray(gup_de).astype(bf),
            # silu(1.702*(x + b)) = silu(1.702*x + 1.702*b); the 1/1.702 glu
            # scale is folded into dwT below.
            "gb": np.ascontiguousarray((1.702 * gupb[e, 0::2]).reshape(NJ, 128).T),
            "ub": np.ascontiguousarray((gupb[e, 1::2] + 1.0).reshape(NJ, 128).T),
            "dwT": np.ascontiguousarray(dw[e].T / np.float32(1.702)).astype(bf),
            "wt": np.ascontiguousarray(rw[:, e].reshape(T // 128, 128).T),
        })
    return in_maps


def _assemble(results, routing_weights, down_proj_bias):
    out = results[0]["outp"].astype(np.float32, copy=True)
    for r in range(1, N_CORES):
        out += results[r]["outp"]
    # routing-weighted down-bias term, summed over experts on the host
    out += np.asarray(routing_weights, dtype=np.float32) @ \
        np.asarray(down_proj_bias, dtype=np.float32)
    return out


def kernel(hidden_states, routing_weights, gate_up_proj, gate_up_proj_bias,
           down_proj, down_proj_bias):
    from concourse import bass_utils
    in_maps = _make_in_maps(hidden_states, routing_weights, gate_up_proj,
                            gate_up_proj_bias, down_proj, down_proj_bias)
    nc = _get_nc()
    try:
        res = bass_utils.run_bass_kernel_spmd(nc, in_maps, core_ids=list(range(N_CORES)))
    except Exception:
        # One retry in case a previous process left a core wedged.
        res = bass_utils.run_bass_kernel_spmd(nc, in_maps, core_ids=list(range(N_CORES)))
    return _assemble(res.results, routing_weights, down_proj_bias)


# revision 6
# speedup vs baseline: 1.3196x; 1.0363x over previous
"""MoE dense all-experts (GPT-OSS Experts forward) on 8 Trainium2 NeuronCores.

Expert-parallel sharding: core e holds expert e's weights and computes its
weighted contribution

    partial_e[t, h] = w[t, e] * ((up + 1) * silu(1.702 * gate) @ down_e.T)

with [gate | up] = hs @ gup_e + bias (the host de-interleaves gup's even/odd
columns so gate/up become contiguous halves). Each core writes its full [T, H]
partial to DRAM; the host sums the 8 partials and adds the routing-weighted
down-bias term (rw @ db) itself, so the device runs no collective at all.

All matmul operands are bf16 (PSUM accumulation stays fp32): bf16 stationary
tiles get the compiler's automatic Fast Weight Load, so LDWEIGHTS (~208 ns at
fp32r) drops to ~27-100 ns and hides under the 512-row moving streams.
End-to-end relative error vs the fp32 reference is ~3.6e-3.

The weight tensors are staged in DRAM as consumption-ordered SBUF images:
gup_p[128, j*1024 + kc*128 + c] so each gate/up j-slice is ONE contiguous
2KB-per-partition-line DMA, issued in exactly the order stage 1 consumes them
(gate j=0 first, hs chunk-0 kc-pieces next, remaining gate, then up, then the
stage-2 weights). The first gate chain starts ~1.5 us after DMA go, and the
k-accumulation is paced by per-slice tile dependencies. hs is fetched in
1024-token pair-tiles (2KB lines); outputs stream out per 512-column half as
soon as each VectorE epilogue (out = psum * w[t]) finishes.
"""
import sys
if '/opt/trn_rl_repo' not in sys.path:
    sys.path.insert(0, '/opt/trn_rl_repo')
import numpy as np
import ml_dtypes

E, H, I, T = 8, 1024, 1024, 4096
N_CORES = 8
TC = 512
NCH = T // TC          # 8 chunks
KC = H // 128          # contraction chunks (H == I == 1024)
NJ = I // 128          # gate/up row tiles

_CACHE = {}


def _build():
    import concourse.bacc as bacc
    import concourse.tile as tile
    import concourse.mybir as mybir
    f32 = mybir.dt.float32
    bf16 = mybir.dt.bfloat16
    AF = mybir.ActivationFunctionType
    ALU = mybir.AluOpType

    nc = bacc.Bacc("TRN2", target_bir_lowering=False, debug=False,
                   enable_asserts=False, num_devices=N_CORES)
    hsT = nc.dram_tensor("hsT", [H, T], bf16, kind="ExternalInput").ap()
    gup = nc.dram_tensor("gup", [128, 2 * NJ * KC * 128], bf16, kind="ExternalInput").ap()
    gb = nc.dram_tensor("gb", [128, NJ], f32, kind="ExternalInput").ap()
    ub = nc.dram_tensor("ub", [128, NJ], f32, kind="ExternalInput").ap()
    dwT = nc.dram_tensor("dwT", [128, 2 * KC * 512], bf16, kind="ExternalInput").ap()
    wt = nc.dram_tensor("wt", [128, T // 128], f32, kind="ExternalInput").ap()
    outp = nc.dram_tensor("outp", [T, H], f32, kind="ExternalOutput").ap()

    with tile.TileContext(nc) as tc_:
        with tc_.tile_pool(name="wpool", bufs=1) as wpool, \
             tc_.tile_pool(name="hpool", bufs=2) as hpool, \
             tc_.tile_pool(name="apool", bufs=2) as apool, \
             tc_.tile_pool(name="spool", bufs=8) as spool, \
             tc_.tile_pool(name="opool", bufs=4) as opool, \
             tc_.tile_pool(name="ps1", bufs=2, space="PSUM") as ps1, \
             tc_.tile_pool(name="ps2", bufs=3, space="PSUM") as ps2:

            gup_r = wpool.tile([128, 2 * NJ * KC * 128], bf16)
            dwT_r = wpool.tile([128, 2 * KC * 512], bf16)
            gb_r = wpool.tile([128, NJ], f32)
            ub_r = wpool.tile([128, NJ], f32)
            w_r = wpool.tile([128, T // 128], f32)

            # DMA order == consumption order. Gate j-slice DMAs are 1024-col
            # contiguous pieces of the prepared layout, so gate chain j waits
            # only on its own slice; hs chunk-0 arrives kc-piece by kc-piece.
            nc.sync.dma_start(gb_r[:], gb[:])
            nc.sync.dma_start(ub_r[:], ub[:])
            nc.sync.dma_start(w_r[:], wt[:])
            hs_pair0 = hpool.tile([128, KC * 2 * TC], bf16, tag="hs")
            hs_pairs = {0: hs_pair0}
            nc.sync.dma_start(gup_r[:, 0:1024], gup[:, 0:1024])  # gate j=0
            for kc in range(KC):                                  # hs chunk 0
                nc.sync.dma_start(hs_pairs[0][:, kc*2*TC : kc*2*TC + TC],
                                  hsT[kc*128:(kc+1)*128, 0:TC])
            for j in range(1, NJ):                                # gate j=1..7
                nc.sync.dma_start(gup_r[:, j*1024:(j+1)*1024],
                                  gup[:, j*1024:(j+1)*1024])
            for kc in range(KC):                                  # hs chunk 1
                nc.sync.dma_start(hs_pairs[0][:, kc*2*TC + TC : (kc+1)*2*TC],
                                  hsT[kc*128:(kc+1)*128, TC:2*TC])
            for j in range(NJ):                                   # up j=0..7
                nc.sync.dma_start(gup_r[:, 8192 + j*1024 : 8192 + (j+1)*1024],
                                  gup[:, 8192 + j*1024 : 8192 + (j+1)*1024])
            for hh in range(2):                                   # down weights
                nc.sync.dma_start(dwT_r[:, hh*4096:(hh+1)*4096],
                                  dwT[:, hh*4096:(hh+1)*4096])

            for c in range(NCH):
                t_off = c * TC
                if c >= 2 and c % 2 == 0:      # prefetch hs pair (c, c+1)
                    hs_pair = hpool.tile([128, KC * 2 * TC], bf16, tag="hs")
                    hs_pairs[c // 2] = hs_pair
                    for kc in range(KC):
                        nc.sync.dma_start(hs_pair[:, kc*2*TC:(kc+1)*2*TC],
                                          hsT[kc*128:(kc+1)*128, t_off:t_off + 2*TC])
                hs_r = hs_pairs[c // 2]
                ho = (c % 2) * TC

                act_r = apool.tile([128, NJ * TC], bf16, tag="act")
                s2s = []
                for j in range(NJ):     # gate pass
                    pg = ps1.tile([128, TC], f32, tag="pg")
                    for kc in range(KC):
                        nc.tensor.matmul(pg[:], gup_r[:, j*1024 + kc*128 : j*1024 + (kc+1)*128],
                                         hs_r[:, kc*2*TC + ho : kc*2*TC + ho + TC],
                                         start=(kc == 0), stop=(kc == KC - 1))
                    s2 = spool.tile([128, TC], f32, tag="s2")
                    nc.scalar.activation(s2[:], pg[:], AF.Silu,
                                         bias=gb_r[:, j:j+1], scale=1.702)
                    s2s.append(s2)
                for j in range(NJ):     # up pass: act = (up + ub + 1) * silu_out
                    pu = ps1.tile([128, TC], f32, tag="pu")
                    for kc in range(KC):
                        nc.tensor.matmul(pu[:], gup_r[:, 8192 + j*1024 + kc*128 : 8192 + j*1024 + (kc+1)*128],
                                         hs_r[:, kc*2*TC + ho : kc*2*TC + ho + TC],
                                         start=(kc == 0), stop=(kc == KC - 1))
                    nc.vector.scalar_tensor_tensor(act_r[:, j*TC:(j+1)*TC], pu[:],
                                                   ub_r[:, j:j+1], s2s[j][:],
                                                   op0=ALU.add, op1=ALU.mult)

                for tt in range(TC // 128):
                    gt = (t_off // 128) + tt
                    wcol = w_r[:, gt:gt+1]
                    for hh in range(2):
                        p2 = ps2.tile([128, 512], f32, tag="p2")
                        for ic in range(KC):
                            nc.tensor.matmul(p2[:], act_r[:, ic*TC + tt*128 : ic*TC + (tt+1)*128],
                                             dwT_r[:, hh*4096 + ic*512 : hh*4096 + (ic+1)*512],
                                             start=(ic == 0), stop=(ic == KC - 1))
                        ot = opool.tile([128, 512], f32, tag="ot")
                        nc.vector.tensor_scalar_mul(ot[:], p2[:], wcol)
                        nc.sync.dma_start(
                            outp[t_off + tt*128 : t_off + (tt+1)*128, hh*512:(hh+1)*512],
                            ot[:])
    nc.compile()
    return nc


def _get_nc():
    if 'nc' not in _CACHE:
        _CACHE['nc'] = _build()
    return _CACHE['nc']


def _make_in_maps(hidden_states, routing_weights, gate_up_proj, gate_up_proj_bias,
                  down_proj, down_proj_bias):
    bf = ml_dtypes.bfloat16
    hs = np.asarray(hidden_states, dtype=np.float32)
    rw = np.asarray(routing_weights, dtype=np.float32)
    gupw = np.asarray(gate_up_proj, dtype=np.float32)
    gupb = np.asarray(gate_up_proj_bias, dtype=np.float32)
    dw = np.asarray(down_proj, dtype=np.float32)
    hsT = np.ascontiguousarray(hs.T).astype(bf)
    in_maps = []
    for e in range(N_CORES):
        g = gupw[e]
        # consumption-ordered SBUF image: [128p, half, j, kc, 128c]
        gup_de = np.stack([g[:, 0::2], g[:, 1::2]])          # [2, H, I]
        gup_p = gup_de.reshape(2, KC, 128, NJ, 128).transpose(2, 0, 3, 1, 4) \
                      .reshape(128, 2 * NJ * KC * 128)
        # stage-2 image: [128p, hh, ic, 512c]; 1/1.702 glu scale folded in
        dwt = (dw[e].T / np.float32(1.702)).reshape(KC, 128, 2, 512) \
                                           .transpose(1, 2, 0, 3).reshape(128, 2 * KC * 512)
        in_maps.append({
            "hsT": hsT,
            "gup": np.ascontiguousarray(gup_p).astype(bf),
            # silu(1.702*(x + b)) = silu(1.702*x + 1.702*b)
            "gb": np.ascontiguousarray((1.702 * gupb[e, 0::2]).reshape(NJ, 128).T),
            "ub": np.ascontiguousarray((gupb[e, 1::2] + 1.0).reshape(NJ, 128).T),
            "dwT": np.ascontiguousarray(dwt).astype(bf),
            "wt": np.ascontiguousarray(rw[:, e].reshape(T // 128, 128).T),
        })
    return in_maps


def _assemble(results, routing_weights, down_proj_bias):
    out = results[0]["outp"].astype(np.float32, copy=True)
    for r in range(1, N_CORES):
        out += results[r]["outp"]
    # routing-weighted down-bias term, summed over experts on the host
    out += np.asarray(routing_weights, dtype=np.float32) @ \
        np.asarray(down_proj_bias, dtype=np.float32)
    return out


def kernel(hidden_states, routing_weights, gate_up_proj, gate_up_proj_bias,
           down_proj, down_proj_bias):
    from concourse import bass_utils
    in_maps = _make_in_maps(hidden_states, routing_weights, gate_up_proj,
                            gate_up_proj_bias, down_proj, down_proj_bias)
    nc = _get_nc()
    try:
        res = bass_utils.run_bass_kernel_spmd(nc, in_maps, core_ids=list(range(N_CORES)))
    except Exception:
        # One retry in case a previous process left a core wedged.
        res = bass_utils.run_bass_kernel_spmd(nc, in_maps, core_ids=list(range(N_CORES)))
    return _assemble(res.results, routing_weights, down_proj_bias)
